# revision 8
# baseline (speedup 1.0000x reference)
"""CausalLocalSGU Trainium2 kernel.

Reference computation (per batch b):
  split x[b] channels -> res (first 1024), gate_in (last 1024)
  per 128-token window block j: z_j = LayerNorm(gate_in_j) * gamma + beta
  gate_out_j[m, c] = sum_n W[h(c), m, n] * [z_{j-1}; z_j][n, c] + bias[h(c), m]
      (W masked causally: keep [m, n] where n <= m + 128; z_{-1} = 0)
  out_j = gate_out_j * res_j

Sharding: 8 cores; core k handles batch k//2, token half k%2 (2048 tokens =
16 window blocks) plus a one-block halo on the left (zeros for even cores).
The LN of the halo block is recomputed locally -> no collectives.

Weights are pre-masked + transposed on host into matmul lhsT layout.
The LN beta term and the bias are folded into one extra K=2 matmul:
  W @ (gamma*z + beta) = W @ (gamma*z) + S[m]*beta[c],  S[m] = sum_n W[m, n]
(with S excluding the first window's columns when the previous block is the
zero pad), plus bias[h, m] * ones[c].

All constants (weights, extras, ones/beta rows) are packed into ONE dram
tensor loaded with a single DMA, and the constants-only extras matmul is
issued first in each PSUM accumulation group, so every PE Matmult needs at
most one sync wait (this walrus rejects matmuls with 2 waits).
"""

import numpy as np

import concourse.bacc as bacc
import concourse.bass as bass
import concourse.tile as tile
from concourse import mybir
from concourse.bass_utils import run_bass_kernel_spmd

F32 = mybir.dt.float32

HEADS = 4
W = 128            # window
DIM = 2048
DOUT = 1024        # dim // 2
DHEAD = DOUT // HEADS  # 256
B = 4
N = 4096
NCORES = 8
BLK_PER_CORE = (N // 2) // W   # 16
LN_EPS = 1e-5

# consts layout (columns of the [128, 3072] consts tensor)
_WT0 = 0            # [128, 1024]  lhsT weights (A_h/B_h interleaved per head)
_EXF0 = 1024        # [2, 512] rows 0..1: bias / S for the first block
_EXR0 = 1536        # [2, 512] rows 0..1: bias / S for the other blocks
_RHSX0 = 2048       # [2, 1024] row 0: ones, row 1: beta
_CONSTS_COLS = 3072

_NC_CACHE: dict = {}
_last_in_maps: list = []


def _build_nc(apply_gamma: bool) -> bass.Bass:
    nc = bacc.Bacc(
        trn_type="TRN2",
        target_bir_lowering=False,
        debug=False,
        num_devices=NCORES,
    )
    nblk = BLK_PER_CORE  # output blocks per core; +1 halo block in x_sh
    x_sh = nc.dram_tensor("x_sh", [(nblk + 1) * W, DIM], F32, kind="ExternalInput").ap()
    consts = nc.dram_tensor("consts", [W, _CONSTS_COLS], F32, kind="ExternalInput").ap()
    if apply_gamma:
        gamma = nc.dram_tensor("gamma", [DOUT], F32, kind="ExternalInput").ap()
    out = nc.dram_tensor("out", [nblk * W, DOUT], F32, kind="ExternalOutput").ap()

    with tile.TileContext(nc) as tc:
        with (
            tc.tile_pool(name="singles", bufs=1) as singles,
            tc.tile_pool(name="xpool", bufs=4) as xpool,
            tc.tile_pool(name="zpool", bufs=3) as zpool,
            tc.tile_pool(name="opool", bufs=3) as opool,
            tc.tile_pool(name="spool", bufs=4) as spool,
            tc.tile_pool(name="ppool", bufs=3, space="PSUM") as ppool,
        ):
            consts_t = singles.tile([W, _CONSTS_COLS], F32)
            nc.sync.dma_start(out=consts_t, in_=consts)
            wt_t = consts_t[:, _WT0 : _WT0 + 2 * HEADS * W]
            exf_t = consts_t[0:2, _EXF0 : _EXF0 + HEADS * W]
            exr_t = consts_t[0:2, _EXR0 : _EXR0 + HEADS * W]
            rhsx_t = consts_t[0:2, _RHSX0 : _RHSX0 + DOUT]

            eps_t = singles.tile([128, 1], F32)
            nc.vector.memset(eps_t, LN_EPS)
            if apply_gamma:
                gamma_t = singles.tile([128, DOUT], F32)
                nc.gpsimd.dma_start(
                    out=gamma_t,
                    in_=bass.AP(
                        tensor=gamma.tensor,
                        offset=gamma.offset,
                        ap=[[0, 128]] + list(gamma.ap),
                    ),
                )

            z_prev = None
            for i in range(nblk + 1):
                xb = xpool.tile([W, DIM], F32, tag="xb")
                if i == 0:
                    # halo block: only the gate half is needed
                    nc.sync.dma_start(out=xb[:, DOUT:], in_=x_sh[0:W, DOUT:])
                else:
                    nc.sync.dma_start(out=xb, in_=x_sh[i * W : (i + 1) * W, :])
                gate = xb[:, DOUT:]

                stats = spool.tile([W, 2, 6], F32)
                nc.vector.bn_stats(out=stats[:, 0], in_=gate[:, :512])
                nc.vector.bn_stats(out=stats[:, 1], in_=gate[:, 512:])
                mv = spool.tile([W, 2], F32)
                nc.vector.bn_aggr(out=mv, in_=stats)
                sd = spool.tile([W, 1], F32)
                nc.scalar.activation(
                    out=sd,
                    in_=mv[:, 1:2],
                    func=mybir.ActivationFunctionType.Sqrt,
                    bias=eps_t,
                )
                rstd = spool.tile([W, 1], F32)
                nc.vector.reciprocal(out=rstd, in_=sd)

                z = zpool.tile([W, DOUT], F32, tag="z")
                nc.vector.tensor_scalar(
                    out=z,
                    in0=gate,
                    scalar1=mv[:, 0:1],
                    scalar2=rstd,
                    op0=mybir.AluOpType.subtract,
                    op1=mybir.AluOpType.mult,
                )
                if apply_gamma:
                    nc.vector.tensor_mul(z, z, gamma_t)

                if i > 0:
                    psum = ppool.tile([W, DOUT], F32, tag="psum")
                    ex_t = exf_t if i == 1 else exr_t
                    for h in range(HEADS):
                        ps = psum[:, h * DHEAD : (h + 1) * DHEAD]
                        zp = z_prev[:, h * DHEAD : (h + 1) * DHEAD]
                        zc = z[:, h * DHEAD : (h + 1) * DHEAD]
                        # constants-only matmul first: absorbs the PSUM-group
                        # wait so data matmuls carry a single (DVE) wait each
                        nc.tensor.matmul(
                            ps,
                            ex_t[:, h * W : (h + 1) * W],
                            rhsx_t[:, h * DHEAD : (h + 1) * DHEAD],
                            start=True,
                            stop=False,
                        )
                        nc.tensor.matmul(
                            ps,
                            wt_t[:, (2 * h) * W : (2 * h + 1) * W],
                            zp,
                            start=False,
                            stop=False,
                        )
                        nc.tensor.matmul(
                            ps,
                            wt_t[:, (2 * h + 1) * W : (2 * h + 2) * W],
                            zc,
                            start=False,
                            stop=True,
                        )
                    ob = opool.tile([W, DOUT], F32, tag="ob")
                    nc.vector.tensor_mul(ob, psum, xb[:, :DOUT])
                    nc.sync.dma_start(out=out[(i - 1) * W : i * W, :], in_=ob)
                z_prev = z
    if not nc.is_finalized():
        nc.finalize()
    return nc


def _host_prep(weight, bias, ln_beta):
    j = np.arange(2 * W)[None, :]
    i_ = np.arange(W)[:, None]
    mask = (j <= i_ + W).astype(np.float32)          # [W, 2W]
    wm = weight * mask[None]                         # [H, W, 2W]
    wT = np.zeros((W, 2 * HEADS, W), dtype=np.float32)
    for h in range(HEADS):
        wT[:, 2 * h] = wm[h, :, :W].T                # A_h: prev-window cols
        wT[:, 2 * h + 1] = wm[h, :, W:].T            # B_h: current-window cols
    wT = wT.reshape(W, 2 * HEADS * W)

    s_full = wm.sum(-1).reshape(HEADS * W)
    s_first = wm[:, :, W:].sum(-1).reshape(HEADS * W)
    bias_flat = bias.reshape(HEADS * W)

    def consts_for(first_has_prev: bool):
        c = np.zeros((W, _CONSTS_COLS), dtype=np.float32)
        c[:, _WT0 : _WT0 + 2 * HEADS * W] = wT
        c[0, _EXF0 : _EXF0 + HEADS * W] = bias_flat
        c[1, _EXF0 : _EXF0 + HEADS * W] = s_full if first_has_prev else s_first
        c[0, _EXR0 : _EXR0 + HEADS * W] = bias_flat
        c[1, _EXR0 : _EXR0 + HEADS * W] = s_full
        c[0, _RHSX0 : _RHSX0 + DOUT] = 1.0
        c[1, _RHSX0 : _RHSX0 + DOUT] = ln_beta
        return c

    return consts_for(False), consts_for(True)


def kernel(x, weight, bias, ln_gamma, ln_beta):
    x = np.ascontiguousarray(x, dtype=np.float32)
    weight = np.asarray(weight, dtype=np.float32)
    bias = np.asarray(bias, dtype=np.float32)
    ln_gamma = np.asarray(ln_gamma, dtype=np.float32)
    ln_beta = np.asarray(ln_beta, dtype=np.float32)

    consts_even, consts_odd = _host_prep(weight, bias, ln_beta)

    apply_gamma = not np.all(ln_gamma == 1.0)
    key = apply_gamma
    if key not in _NC_CACHE:
        _NC_CACHE[key] = _build_nc(apply_gamma)
    nc = _NC_CACHE[key]

    half = N // 2
    in_maps = []
    for k in range(NCORES):
        bk, hk = k // 2, k % 2
        x_half = x[bk, hk * half : (hk + 1) * half]  # [2048, 2048]
        if hk == 0:
            halo = np.zeros((W, DIM), dtype=np.float32)
        else:
            halo = x[bk, half - W : half]
        x_sh = np.ascontiguousarray(np.concatenate([halo, x_half], axis=0))
        m = {
            "x_sh": x_sh,
            "consts": consts_odd if hk == 1 else consts_even,
        }
        if apply_gamma:
            m["gamma"] = ln_gamma
        in_maps.append(m)

    global _last_in_maps
    _last_in_maps = in_maps

    res = run_bass_kernel_spmd(nc, in_maps, list(range(NCORES)))

    out = np.empty((B, N, DOUT), dtype=np.float32)
    for k in range(NCORES):
        bk, hk = k // 2, k % 2
        out[bk, hk * half : (hk + 1) * half] = res.results[k]["out"]
    return out


# revision 9
# speedup vs baseline: 1.5646x; 1.5646x over previous
"""CausalLocalSGU Trainium2 kernel.

Reference computation (per batch b):
  split x[b] channels -> res (first 1024), gate_in (last 1024)
  per 128-token window block j: z_j = LayerNorm(gate_in_j) * gamma + beta
  gate_out_j[m, c] = sum_n W[h(c), m, n] * [z_{j-1}; z_j][n, c] + bias[h(c), m]
      (W masked causally: keep [m, n] where n <= m + 128; z_{-1} = 0)
  out_j = gate_out_j * res_j

Sharding: 8 cores; core k handles batch k//2, token half k%2 (2048 tokens =
16 window blocks) plus a one-block halo on the left (zeros for even cores).
The LN of the halo block is recomputed locally -> no collectives.

Device pipeline per block (fast path: gamma==1, beta==0):
  DMA gate (bf16) + res (fp32) -> bn_stats/bn_aggr (DVE) -> rstd (ACT sqrt +
  DVE recip) -> normalize on ACT (Identity, scale=rstd, bias=-mu*rstd) into a
  bf16 z tile -> 8 bf16 matmuls (prev/current window per head, causal mask +
  transpose pre-applied to the weights on host) -> ACT Identity adds the
  per-head bias from PSUM -> DVE multiply by res -> DMA out.

The gate half is cast to bf16 on the host: the einsum term it feeds
contributes ~7e-5 of the output magnitude (weights ~1e-5), so bf16 there
perturbs the output by ~3e-7 relative while halving LN traffic and PE time.

General path (gamma != 1 or beta != 0) additionally multiplies z by gamma
(DVE) and folds beta+bias through an fp32 K=2 matmul:
  W @ (gamma*z + beta) = W @ (gamma*z) + S[m]*beta[c],  S[m] = sum_n W[m, n]
(S excludes the first window's columns when the previous block is zero pad).
"""

import ml_dtypes
import numpy as np

import concourse.bacc as bacc
import concourse.bass as bass
import concourse.tile as tile
from concourse import mybir
from concourse.bass_utils import run_bass_kernel_spmd

F32 = mybir.dt.float32
BF16 = mybir.dt.bfloat16

HEADS = 4
W = 128            # window
DIM = 2048
DOUT = 1024        # dim // 2
DHEAD = DOUT // HEADS  # 256
B = 4
N = 4096
NCORES = 8
BLK_PER_CORE = (N // 2) // W   # 16
LN_EPS = 1e-5

# fp32 consts layout (columns of the [128, 2056] consts tensor)
_BIAS0 = 0          # 4 cols: bias[h, :] as per-partition columns
_EXF0 = 8           # [2, 512] rows 0..1: bias / S for the first block
_EXR0 = 520         # [2, 512] rows 0..1: bias / S for the other blocks
_RHSX0 = 1032       # [2, 1024] row 0: ones, row 1: beta
_CONSTS_COLS = 2056

_NC_CACHE: dict = {}
_last_in_maps: list = []


def _build_nc(general: bool) -> bass.Bass:
    nc = bacc.Bacc(
        trn_type="TRN2",
        target_bir_lowering=False,
        debug=False,
        num_devices=NCORES,
    )
    nblk = BLK_PER_CORE  # output blocks per core; +1 halo block for gate
    res_sh = nc.dram_tensor("res_sh", [nblk * W, DOUT], F32, kind="ExternalInput").ap()
    gate_sh = nc.dram_tensor(
        "gate_sh", [(nblk + 1) * W, DOUT], BF16, kind="ExternalInput"
    ).ap()
    consts = nc.dram_tensor("consts", [W, _CONSTS_COLS], F32, kind="ExternalInput").ap()
    consts_bf = nc.dram_tensor(
        "consts_bf", [W, 2 * HEADS * W], BF16, kind="ExternalInput"
    ).ap()
    if general:
        gamma = nc.dram_tensor("gamma", [DOUT], F32, kind="ExternalInput").ap()
    out = nc.dram_tensor("out", [nblk * W, DOUT], F32, kind="ExternalOutput").ap()

    ident = mybir.ActivationFunctionType.Identity

    with tile.TileContext(nc) as tc:
        with (
            tc.tile_pool(name="singles", bufs=1) as singles,
            tc.tile_pool(name="gpool", bufs=4) as gpool,
            tc.tile_pool(name="rpool", bufs=4) as rpool,
            tc.tile_pool(name="zpool", bufs=3) as zpool,
            tc.tile_pool(name="opool", bufs=3) as opool,
            tc.tile_pool(name="spool", bufs=4) as spool,
            tc.tile_pool(name="ppool", bufs=3, space="PSUM") as ppool,
        ):
            consts_t = singles.tile([W, _CONSTS_COLS], F32)
            nc.sync.dma_start(out=consts_t, in_=consts)
            wt_t = singles.tile([W, 2 * HEADS * W], BF16)
            nc.sync.dma_start(out=wt_t, in_=consts_bf)
            exf_t = consts_t[0:2, _EXF0 : _EXF0 + HEADS * W]
            exr_t = consts_t[0:2, _EXR0 : _EXR0 + HEADS * W]
            rhsx_t = consts_t[0:2, _RHSX0 : _RHSX0 + DOUT]

            eps_t = singles.tile([128, 1], F32)
            nc.vector.memset(eps_t, LN_EPS)
            if general:
                gamma_t = singles.tile([128, DOUT], F32)
                nc.gpsimd.dma_start(
                    out=gamma_t,
                    in_=bass.AP(
                        tensor=gamma.tensor,
                        offset=gamma.offset,
                        ap=[[0, 128]] + list(gamma.ap),
                    ),
                )

            z_prev = None
            for i in range(nblk + 1):
                gate = gpool.tile([W, DOUT], BF16, tag="gate")
                nc.sync.dma_start(out=gate, in_=gate_sh[i * W : (i + 1) * W, :])
                if i > 0:
                    res = rpool.tile([W, DOUT], F32, tag="res")
                    nc.sync.dma_start(
                        out=res, in_=res_sh[(i - 1) * W : i * W, :]
                    )

                stats = spool.tile([W, 2, 6], F32)
                nc.vector.bn_stats(out=stats[:, 0], in_=gate[:, :512])
                nc.vector.bn_stats(out=stats[:, 1], in_=gate[:, 512:])
                mv = spool.tile([W, 2], F32)
                nc.vector.bn_aggr(out=mv, in_=stats)
                sd = spool.tile([W, 1], F32)
                nc.scalar.activation(
                    out=sd,
                    in_=mv[:, 1:2],
                    func=mybir.ActivationFunctionType.Sqrt,
                    bias=eps_t,
                )
                rstd = spool.tile([W, 1], F32)
                nc.vector.reciprocal(out=rstd, in_=sd)
                negmu = spool.tile([W, 1], F32)
                nc.vector.tensor_scalar(
                    out=negmu,
                    in0=mv[:, 0:1],
                    scalar1=rstd,
                    scalar2=-1.0,
                    op0=mybir.AluOpType.mult,
                    op1=mybir.AluOpType.mult,
                )

                z = zpool.tile([W, DOUT], BF16, tag="z")
                # z = (gate - mu) * rstd   via ACT: gate*rstd + (-mu*rstd)
                nc.scalar.activation(
                    out=z, in_=gate, func=ident, bias=negmu, scale=rstd
                )
                if general:
                    nc.vector.tensor_mul(z, z, gamma_t)

                if i > 0:
                    psum = ppool.tile([W, DOUT], F32, tag="psum")
                    ex_t = exf_t if i == 1 else exr_t
                    for h in range(HEADS):
                        ps = psum[:, h * DHEAD : (h + 1) * DHEAD]
                        zp = z_prev[:, h * DHEAD : (h + 1) * DHEAD]
                        zc = z[:, h * DHEAD : (h + 1) * DHEAD]
                        if general:
                            nc.tensor.matmul(
                                ps,
                                ex_t[:, h * W : (h + 1) * W],
                                rhsx_t[:, h * DHEAD : (h + 1) * DHEAD],
                                start=True,
                                stop=False,
                            )
                        nc.tensor.matmul(
                            ps,
                            wt_t[:, (2 * h) * W : (2 * h + 1) * W],
                            zp,
                            start=not general,
                            stop=False,
                        )
                        nc.tensor.matmul(
                            ps,
                            wt_t[:, (2 * h + 1) * W : (2 * h + 2) * W],
                            zc,
                            start=False,
                            stop=True,
                        )
                    ob = opool.tile([W, DOUT], F32, tag="ob")
                    if general:
                        # bias/beta already folded in via the extras matmul
                        nc.vector.tensor_mul(ob, psum, res)
                    else:
                        for h in range(HEADS):
                            nc.scalar.activation(
                                out=ob[:, h * DHEAD : (h + 1) * DHEAD],
                                in_=psum[:, h * DHEAD : (h + 1) * DHEAD],
                                func=ident,
                                bias=consts_t[:, _BIAS0 + h : _BIAS0 + h + 1],
                                scale=1.0,
                            )
                        nc.vector.tensor_mul(ob, ob, res)
                    nc.sync.dma_start(out=out[(i - 1) * W : i * W, :], in_=ob)
                z_prev = z
    if not nc.is_finalized():
        nc.finalize()
    return nc


def _host_prep(weight, bias, ln_beta):
    j = np.arange(2 * W)[None, :]
    i_ = np.arange(W)[:, None]
    mask = (j <= i_ + W).astype(np.float32)          # [W, 2W]
    wm = weight * mask[None]                         # [H, W, 2W]
    wT = np.zeros((W, 2 * HEADS, W), dtype=np.float32)
    for h in range(HEADS):
        wT[:, 2 * h] = wm[h, :, :W].T                # A_h: prev-window cols
        wT[:, 2 * h + 1] = wm[h, :, W:].T            # B_h: current-window cols
    wT = wT.reshape(W, 2 * HEADS * W)

    s_full = wm.sum(-1).reshape(HEADS * W)
    s_first = wm[:, :, W:].sum(-1).reshape(HEADS * W)
    bias_flat = bias.reshape(HEADS * W)

    def consts_for(first_has_prev: bool):
        c = np.zeros((W, _CONSTS_COLS), dtype=np.float32)
        for h in range(HEADS):
            c[:, _BIAS0 + h] = bias[h]
        c[0, _EXF0 : _EXF0 + HEADS * W] = bias_flat
        c[1, _EXF0 : _EXF0 + HEADS * W] = s_full if first_has_prev else s_first
        c[0, _EXR0 : _EXR0 + HEADS * W] = bias_flat
        c[1, _EXR0 : _EXR0 + HEADS * W] = s_full
        c[0, _RHSX0 : _RHSX0 + DOUT] = 1.0
        c[1, _RHSX0 : _RHSX0 + DOUT] = ln_beta
        return c

    consts_bf = np.ascontiguousarray(wT.astype(ml_dtypes.bfloat16))
    return consts_for(False), consts_for(True), consts_bf


def kernel(x, weight, bias, ln_gamma, ln_beta):
    x = np.ascontiguousarray(x, dtype=np.float32)
    weight = np.asarray(weight, dtype=np.float32)
    bias = np.asarray(bias, dtype=np.float32)
    ln_gamma = np.asarray(ln_gamma, dtype=np.float32)
    ln_beta = np.asarray(ln_beta, dtype=np.float32)

    consts_even, consts_odd, consts_bf = _host_prep(weight, bias, ln_beta)

    general = not (np.all(ln_gamma == 1.0) and np.all(ln_beta == 0.0))
    if general not in _NC_CACHE:
        _NC_CACHE[general] = _build_nc(general)
    nc = _NC_CACHE[general]

    half = N // 2
    gate_bf = np.ascontiguousarray(x[:, :, DOUT:]).astype(ml_dtypes.bfloat16)
    in_maps = []
    for k in range(NCORES):
        bk, hk = k // 2, k % 2
        res_sh = np.ascontiguousarray(x[bk, hk * half : (hk + 1) * half, :DOUT])
        if hk == 0:
            halo = np.zeros((W, DOUT), dtype=ml_dtypes.bfloat16)
        else:
            halo = gate_bf[bk, half - W : half]
        gate_sh = np.ascontiguousarray(
            np.concatenate([halo, gate_bf[bk, hk * half : (hk + 1) * half]], axis=0)
        )
        m = {
            "res_sh": res_sh,
            "gate_sh": gate_sh,
            "consts": consts_odd if hk == 1 else consts_even,
            "consts_bf": consts_bf,
        }
        if general:
            m["gamma"] = ln_gamma
        in_maps.append(m)

    global _last_in_maps
    _last_in_maps = in_maps

    res = run_bass_kernel_spmd(nc, in_maps, list(range(NCORES)))

    out = np.empty((B, N, DOUT), dtype=np.float32)
    for k in range(NCORES):
        bk, hk = k // 2, k % 2
        out[bk, hk * half : (hk + 1) * half] = res.results[k]["out"]
    return out
